# revision 4
# baseline (speedup 1.0000x reference)
"""KANLinear Trainium2 kernel — fp8 DoubleRow + bf16 hybrid matmul.

Math (reference):
    xc     = clip(x, -1, 1)                                  # (N, in)
    base   = silu(xc) @ scale_base.T                         # (N, out)
    b0=1, b1=xc, b_k = 2*xc*b_{k-1} - 1
    spline[n,o] = sum_{i,k} scale_spline[o,i]*coeff[o,i,k]*b_k(xc[n,i])
    out    = base + spline + sum_i base_bias[o,i]

Device formulation: one big matmul over 8 features per input channel.
With t = clip(2x, -2, 2) and the shifted basis s_k = b_k + 1 (k>=2):
    s_2 = (t*0.5)*t,  s_{k+1} = (s_k - 1)*t     # one fused DVE op each
    features = [t, silu(t/2), s_2..s_7]         # 8 per input channel
    out[n,o] = sum_{i,f} F[f,i,n] * W[(f,i), o] + bias[o]
where W folds scale_base / scale_spline*coeff (and the s-shift and the
t=2x scaling), bias folds the k=0 term, the s-shift and base_bias.

Precision/perf split: rows {t, silu, s2..s5} carry ~7% of the output
variance -> fp8 e4m3 with MatmulPerfMode.DoubleRow (2 contraction rows
per PE pass); rows {s6, s7} carry ~93% -> bf16.  Their clip-atom values
(+-2, 6, -10) are exactly representable in e4m3, so quantization error
stays ~1e-2 (gate is 2e-2).

Sharding: data-parallel over the 8192 tokens -> 1024 tokens per core
(core b gets batch b).  Each core computes its full [1024, 512] output
block; no collectives.  Host does layout transforms + the tiny bias add.
"""

import os

import numpy as np
import ml_dtypes

import concourse.bass as bass
import concourse.tile as tile
from concourse import bacc, mybir
from concourse import bass_utils

B, S, IN_F, OUT_F, K = 8, 1024, 512, 512, 8
NCORES = 8
N_PER = (B * S) // NCORES          # 1024 tokens per core
ICHUNKS = IN_F // 128              # 4 input-channel chunks
NPAIR = 3                          # fp8 DoubleRow pairs per ichunk
PAIRS = ICHUNKS * NPAIR            # 12 fp8 pair chunks (256 rows each)
NBF = 2                            # bf16 rows per ichunk (s6, s7)
BCHUNKS = ICHUNKS * NBF            # 8 bf16 chunks of 128 rows
OT = OUT_F // 128                  # 4 output tiles
NH = N_PER // 512                  # 2 moving halves

ALU = mybir.AluOpType
ACT_FN = mybir.ActivationFunctionType
DR = mybir.MatmulPerfMode.DoubleRow

F8 = mybir.dt.float8e4
BF = mybir.dt.bfloat16
NP_F8 = ml_dtypes.float8_e4m3
NP_BF = ml_dtypes.bfloat16

MM_DTYPE = os.environ.get("KERNEL_MM_DTYPE", "hybrid")

_compiled = {}


def _build(mm_dtype: str, repeats: int = 1):
    nc = bacc.Bacc(
        "TRN2", target_bir_lowering=False, debug=False, enable_asserts=False
    )
    t_in = nc.dram_tensor(
        "t_in", [IN_F, N_PER], mybir.dt.float32, kind="ExternalInput"
    ).ap()
    # fp8 pairs: pair p covers 2 feature-chunks; row layout [2, 512] per part
    w8 = nc.dram_tensor(
        "w8", [PAIRS * 128, 2 * OUT_F], F8, kind="ExternalInput"
    ).ap()
    w16 = nc.dram_tensor(
        "w16", [BCHUNKS * 128, OUT_F], BF, kind="ExternalInput"
    ).ap()
    # transposed output: [out_features, tokens]; host transposes back
    out = nc.dram_tensor(
        "out", [OUT_F, N_PER], mybir.dt.float32, kind="ExternalOutput"
    ).ap()

    with tile.TileContext(nc) as tc:
        with (
            tc.tile_pool(name="xp", bufs=2) as xp,
            tc.tile_pool(name="tp", bufs=2) as tp,
            tc.tile_pool(name="fp", bufs=4) as fp,
            tc.tile_pool(name="f8p", bufs=4) as f8p,
            tc.tile_pool(name="wp", bufs=6) as wp,
            tc.tile_pool(name="wbp", bufs=4) as wbp,
            tc.tile_pool(name="op", bufs=2) as op,
            tc.tile_pool(name="pp", bufs=1, space="PSUM") as pp,
        ):
            for rep in range(repeats):
                # psum[ot] holds out.T rows ot*128..+128: [128 o, 1024 tok]
                psums = [
                    pp.tile([128, N_PER], mybir.dt.float32, tag=f"ps{ot}",
                            name=f"ps{ot}_{rep}")
                    for ot in range(OT)
                ]
                xts = []
                w8ts = []
                w16ts = []

                def emit_x(c, rep=rep, xts=xts):
                    xt = xp.tile([128, N_PER], mybir.dt.float32, tag="x",
                                 name=f"x{c}_{rep}")
                    nc.sync.dma_start(out=xt, in_=t_in[c * 128:(c + 1) * 128, :])
                    xts.append(xt)

                # stream weights; interleave x loads so activations don't
                # queue behind the whole weight stream
                emit_x(0)
                for c in range(ICHUNKS):
                    for p in range(NPAIR):
                        pi = c * NPAIR + p
                        wt = wp.tile([128, 2, OUT_F], F8, tag="w8",
                                     name=f"w8_{pi}_{rep}")
                        nc.sync.dma_start(
                            out=wt, in_=w8[pi * 128:(pi + 1) * 128, :])
                        w8ts.append(wt)
                    for b in range(NBF):
                        bi = c * NBF + b
                        wt = wbp.tile([128, OUT_F], BF, tag="w16",
                                      name=f"w16_{bi}_{rep}")
                        nc.sync.dma_start(
                            out=wt, in_=w16[bi * 128:(bi + 1) * 128, :])
                        w16ts.append(wt)
                    if c + 1 < ICHUNKS:
                        emit_x(c + 1)

                NGRP = ICHUNKS * (NPAIR + NBF)   # 20 contraction groups

                def mm_dr(pairf, pi, gi, rep=rep):
                    # lhsT = [128, 2, 128] fp8 stationary; rhs = [128, 2, 512]
                    wt = w8ts[pi]
                    for ot in range(OT):
                        for h in range(NH):
                            nc.tensor.matmul(
                                psums[ot][:, h * 512:(h + 1) * 512],
                                wt[:, :, ot * 128:(ot + 1) * 128],
                                pairf[:, :, h * 512:(h + 1) * 512],
                                start=(gi == 0),
                                stop=(gi == NGRP - 1),
                                perf_mode=DR,
                            )

                def mm_bf(feat, bi, gi, rep=rep):
                    wt = w16ts[bi]
                    for ot in range(OT):
                        for h in range(NH):
                            nc.tensor.matmul(
                                psums[ot][:, h * 512:(h + 1) * 512],
                                wt[:, ot * 128:(ot + 1) * 128],
                                feat[:, h * 512:(h + 1) * 512],
                                start=(gi == 0),
                                stop=(gi == NGRP - 1),
                            )

                for c in range(ICHUNKS):
                    xt = xts[c]
                    # t = clip(2x, -2, 2)   (host sends 2x)
                    t = tp.tile([128, N_PER], BF, tag="t", name=f"t{c}_{rep}")
                    nc.vector.tensor_scalar(
                        out=t, in0=xt, scalar1=2.0, scalar2=-2.0,
                        op0=ALU.min, op1=ALU.max,
                    )
                    # pair 0: (t, silu)
                    pair0 = f8p.tile([128, 2, N_PER], F8, tag="f8",
                                     name=f"p0_{c}_{rep}")
                    nc.scalar.activation(out=pair0[:, 0, :], in_=t,
                                         func=ACT_FN.Copy)
                    sg = fp.tile([128, N_PER], BF, tag="f", name=f"sg{c}_{rep}")
                    nc.scalar.activation(out=sg, in_=t, func=ACT_FN.Sigmoid,
                                         scale=0.5)
                    # silu(t/2) = (t*0.5)*sigmoid(t/2), direct fp8 out
                    nc.vector.scalar_tensor_tensor(
                        out=pair0[:, 1, :], in0=t, scalar=0.5, in1=sg,
                        op0=ALU.mult, op1=ALU.mult,
                    )
                    mm_dr(pair0, c * NPAIR + 0, c * 5 + 0)
                    # s2 = (t*0.5)*t ;  s_{k+1} = (s_k - 1)*t
                    pair1 = f8p.tile([128, 2, N_PER], F8, tag="f8",
                                     name=f"p1_{c}_{rep}")
                    s2 = fp.tile([128, N_PER], BF, tag="f", name=f"s2_{c}_{rep}")
                    nc.vector.scalar_tensor_tensor(
                        out=s2, in0=t, scalar=0.5, in1=t,
                        op0=ALU.mult, op1=ALU.mult,
                    )
                    nc.scalar.activation(out=pair1[:, 0, :], in_=s2,
                                         func=ACT_FN.Copy)
                    s3 = fp.tile([128, N_PER], BF, tag="f", name=f"s3_{c}_{rep}")
                    nc.vector.scalar_tensor_tensor(
                        out=s3, in0=s2, scalar=-1.0, in1=t,
                        op0=ALU.add, op1=ALU.mult,
                    )
                    nc.scalar.activation(out=pair1[:, 1, :], in_=s3,
                                         func=ACT_FN.Copy)
                    mm_dr(pair1, c * NPAIR + 1, c * 5 + 1)
                    pair2 = f8p.tile([128, 2, N_PER], F8, tag="f8",
                                     name=f"p2_{c}_{rep}")
                    s4 = fp.tile([128, N_PER], BF, tag="f", name=f"s4_{c}_{rep}")
                    nc.vector.scalar_tensor_tensor(
                        out=s4, in0=s3, scalar=-1.0, in1=t,
                        op0=ALU.add, op1=ALU.mult,
                    )
                    nc.scalar.activation(out=pair2[:, 0, :], in_=s4,
                                         func=ACT_FN.Copy)
                    s5 = fp.tile([128, N_PER], BF, tag="f", name=f"s5_{c}_{rep}")
                    nc.vector.scalar_tensor_tensor(
                        out=s5, in0=s4, scalar=-1.0, in1=t,
                        op0=ALU.add, op1=ALU.mult,
                    )
                    nc.scalar.activation(out=pair2[:, 1, :], in_=s5,
                                         func=ACT_FN.Copy)
                    mm_dr(pair2, c * NPAIR + 2, c * 5 + 2)
                    # s6, s7 in bf16
                    s6 = fp.tile([128, N_PER], BF, tag="f", name=f"s6_{c}_{rep}")
                    nc.vector.scalar_tensor_tensor(
                        out=s6, in0=s5, scalar=-1.0, in1=t,
                        op0=ALU.add, op1=ALU.mult,
                    )
                    mm_bf(s6, c * NBF + 0, c * 5 + 3)
                    s7 = fp.tile([128, N_PER], BF, tag="f", name=f"s7_{c}_{rep}")
                    nc.vector.scalar_tensor_tensor(
                        out=s7, in0=s6, scalar=-1.0, in1=t,
                        op0=ALU.add, op1=ALU.mult,
                    )
                    mm_bf(s7, c * NBF + 1, c * 5 + 4)

                for ot_i in range(OT):
                    osb = op.tile([128, N_PER], mybir.dt.float32, tag="o",
                                  name=f"o{ot_i}_{rep}")
                    nc.vector.tensor_copy(out=osb, in_=psums[ot_i][:, :])
                    nc.sync.dma_start(
                        out=out[ot_i * 128:(ot_i + 1) * 128, :], in_=osb
                    )

    nc.compile()
    return nc


def _get_nc(mm_dtype: str, repeats: int = 1):
    key = (mm_dtype, repeats)
    if key not in _compiled:
        _compiled[key] = _build(mm_dtype, repeats)
    return _compiled[key]


def _prep_weights(coeff, scale_base, scale_spline, base_bias, mm_dtype: str):
    """Fold scales/basis-shift into per-feature weight rows + bias vector.

    Feature order: f0=t (=2*xc), f1=silu(xc), f2..f7 = s_k = b_k+1.
    fp8 pairs per ichunk: (f0,f1), (f2,f3), (f4,f5); bf16 rows: f6, f7.
    """
    w_spl = (scale_spline.astype(np.float64)[:, :, None]
             * coeff.astype(np.float64))                      # (o, i, k)
    Wf = np.empty((ICHUNKS, K, 128, OUT_F), np.float64)
    for c in range(ICHUNKS):
        sl = slice(c * 128, (c + 1) * 128)
        Wf[c, 0] = w_spl[:, sl, 1].T * 0.5                    # feature t = 2*xc
        Wf[c, 1] = scale_base.astype(np.float64).T[sl]        # silu(t/2)=silu(xc)
        for k in range(2, K):
            Wf[c, k] = w_spl[:, sl, k].T                      # feature s_k=b_k+1
    # bias: k=0 term (b0=1), minus the +1 shift of s_2..s_7, plus base_bias
    bias = (w_spl[:, :, 0] - w_spl[:, :, 2:].sum(-1)).sum(1) \
        + base_bias.astype(np.float64).sum(1)

    # pack fp8 pairs: [ICHUNKS, 3 pairs, 128 part, 2, OUT] -> [PAIRS*128, 2*OUT]
    W8 = np.empty((ICHUNKS, NPAIR, 128, 2, OUT_F), np.float64)
    for c in range(ICHUNKS):
        for p in range(NPAIR):
            W8[c, p, :, 0, :] = Wf[c, 2 * p + 0]
            W8[c, p, :, 1, :] = Wf[c, 2 * p + 1]
    W8 = np.ascontiguousarray(
        W8.reshape(PAIRS * 128, 2 * OUT_F)).astype(NP_F8)
    # bf16 rows: f6, f7 per ichunk
    W16 = np.empty((ICHUNKS, NBF, 128, OUT_F), np.float64)
    for c in range(ICHUNKS):
        W16[c, 0] = Wf[c, 6]
        W16[c, 1] = Wf[c, 7]
    W16 = np.ascontiguousarray(
        W16.reshape(BCHUNKS * 128, OUT_F)).astype(NP_BF)
    return W8, W16, bias.astype(np.float32)


def _make_in_maps(x, W8, W16):
    xr = np.asarray(x, dtype=np.float32).reshape(NCORES, N_PER, IN_F)
    in_maps = []
    for b in range(NCORES):
        t_b = np.ascontiguousarray((2.0 * xr[b]).T.astype(np.float32))
        in_maps.append({"t_in": t_b, "w8": W8, "w16": W16})
    return in_maps


def kernel(x, coeff, scale_base, scale_spline, base_bias):
    x = np.asarray(x, dtype=np.float32)
    coeff = np.asarray(coeff, dtype=np.float32)
    scale_base = np.asarray(scale_base, dtype=np.float32)
    scale_spline = np.asarray(scale_spline, dtype=np.float32)
    base_bias = np.asarray(base_bias, dtype=np.float32)
    mm_dtype = MM_DTYPE
    nc = _get_nc(mm_dtype)
    W8, W16, bias = _prep_weights(coeff, scale_base, scale_spline, base_bias,
                                  mm_dtype)
    in_maps = _make_in_maps(x, W8, W16)

    trace = bool(int(os.environ.get("KERNEL_TRACE", "0")))
    res = bass_utils.run_bass_kernel_spmd(
        nc, in_maps, core_ids=list(range(NCORES)), trace=trace
    )
    global LAST_RESULT
    LAST_RESULT = res
    out = np.stack([res.results[b]["out"].T for b in range(NCORES)], axis=0)
    out = out + bias[None, None, :]
    return out.reshape(B, S, OUT_F).astype(np.float32)


LAST_RESULT = None
